# revision 60
# baseline (speedup 1.0000x reference)
"""Trainium2 Bass kernel for CHMSA (cross-covariance multi-head self-attention
with a ConvNorm qkv stem).

Problem (hardcoded):
  x         [16, 64, 64, 256] f32
  dw_kernel [3, 3, 1, 256]    depthwise 3x3, SAME
  bn_gamma/bn_beta [256]      per-channel affine after dwconv
  pw_kernel [256, 768]        1x1 conv -> qkv
  q_bias/v_bias [256]         qkv bias = concat([q_bias, 0, v_bias])
  scale     [8,1,1]           per-head logit scale, s = exp(min(scale, ln 100))
  proj_w    [256, 256], proj_b [256]

Sharding: pure data-parallel over batch: 16 images / 8 cores = 2 images/core.
No collectives.

Per-core dataflow (per image, N = 4096 tokens, C = 256):
  1. DMA x token-major [128,256] tiles; PE-transpose -> x^T channel-major.
  2. dwconv: 9 diagonal matmuls per PSUM tile (channel-major), gamma folded
     into the diagonal weights, beta added at eviction. SAME padding via
     ragged per-tap APs (center tap first with start=True).
  3. qkv: q,k token-major (lhsT = y^T columns), v channel-major.
  4. l2-normalize: squares on ACT, grouped reduce on DVE, then a single
     per-token weight w = s_h * rsqrt(max(sq_q,eps)*max(sq_k,eps)) applied to
     q only (mathematically identical to normalizing q and k separately and
     scaling by s).
  5. attn = qs^T k per 4-head group ([128,128] gram, diagonal blocks used),
     softmax on packed [128,32] tiles, 32x32 PE transposes -> attn^T (bf16).
  6. out_cm = attn^T-weighted v (channel-major, bf16), proj back to
     token-major, DMA out.
"""

import math

import numpy as np
import ml_dtypes

import concourse.bass as bass
import concourse.mybir as mybir
import concourse.tile as tile
from concourse import bacc
from concourse.bass_utils import run_bass_kernel_spmd

F32 = mybir.dt.float32
F32R = mybir.dt.float32r
BF16 = mybir.dt.bfloat16
AF = mybir.ActivationFunctionType
ALU = mybir.AluOpType

B, H, W, C = 16, 64, 64, 256
N = H * W              # 4096 tokens per image
HEADS = 8
HD = C // HEADS        # 32
NCORES = 8
IMGS = B // NCORES     # 2 images per core
NCH = C // 128         # 2 channel chunks
LOG_MAX_SCALE = float(np.log(100.0))
L2_EPS = 1e-12

# dwconv tap offsets (dh, dw), center first so it can carry start=True with
# full-tile coverage; the ragged edge taps then accumulate.
TAPS = [(0, 0), (-1, -1), (-1, 0), (-1, 1), (0, -1), (0, 1), (1, -1), (1, 0), (1, 1)]

HBLK = 8               # h-rows per dwconv psum tile -> free dim 8*64 = 512
NBLK_G = N // 128


def _r(ap):
    """View an fp32 AP as float32r for full-rate PE matmuls."""
    return ap.bitcast(F32R)


def _build_program(consts, add_qbias, add_pbias, reps=1, debug_taps=False):
    nc = bacc.Bacc()

    x_dr = nc.dram_tensor("x", [IMGS, N, C], F32, kind="ExternalInput")
    out_dr = nc.dram_tensor("out", [IMGS, N, C], F32, kind="ExternalOutput")
    assert not debug_taps

    diag_dr = nc.inline_tensor(consts["diag"], "cdiag")        # [128, NCH, 9, 128]
    pwqk_dr = nc.inline_tensor(consts["pwqk"], "cpwqk")        # [128, NCH, 512]
    pwv_dr = nc.inline_tensor(consts["pwv"], "cpwv")           # [128, NCH, NCH, 128]
    projw_dr = nc.inline_tensor(consts["projw"], "cprojw")     # [128, NCH, 256] bf16
    ident_dr = nc.inline_tensor(consts["ident"], "cident")     # [128, 128]
    beta_dr = nc.inline_tensor(consts["beta"], "cbeta")        # [128, NCH]
    vb_dr = nc.inline_tensor(consts["vb"], "cvb")              # [128, NCH]
    srep_dr = nc.inline_tensor(consts["srep"], "csrep")        # [128, 32*8]
    if add_qbias:
        qb_dr = nc.inline_tensor(consts["qb"], "cqb")          # [128, 256]
    if add_pbias:
        pb_dr = nc.inline_tensor(consts["pb"], "cpb")          # [128, 256] bf16-> f32

    with tile.TileContext(nc) as tc:
        with (
            tc.tile_pool(name="singles", bufs=1) as singles,
            tc.tile_pool(name="xstage", bufs=3) as xstage,
            tc.tile_pool(name="xt", bufs=1) as xt_pool,
            tc.tile_pool(name="img_big", bufs=1) as img_pool,
            tc.tile_pool(name="qs", bufs=3) as qs_pool,
            tc.tile_pool(name="small", bufs=3) as small,
            tc.tile_pool(name="sq", bufs=3) as sq_pool,
            tc.tile_pool(name="ostage", bufs=2) as ostage,
            tc.tile_pool(name="ps_mm", bufs=4, space="PSUM") as ps_mm,
            tc.tile_pool(name="ps_c", bufs=2, space="PSUM") as ps_c,
            tc.tile_pool(name="ps_attn", bufs=2, space="PSUM") as ps_attn,
        ):
            # ---- constants into SBUF ----
            ident_sb = singles.tile([128, 128], F32)
            from concourse.masks import make_identity
            make_identity(nc, ident_sb[:])
            diag_sb = singles.tile([128, NCH, 9, 128], F32)
            nc.gpsimd.dma_start(diag_sb[:], diag_dr[:])
            pwqk_sb = singles.tile([128, NCH, 512], F32)
            nc.gpsimd.dma_start(pwqk_sb[:], pwqk_dr[:])
            pwv_sb = singles.tile([128, NCH, NCH, 128], F32)
            nc.gpsimd.dma_start(pwv_sb[:], pwv_dr[:])
            projw_sb = singles.tile([128, NCH, 256], BF16)
            nc.gpsimd.dma_start(projw_sb[:], projw_dr[:])
            beta_sb = singles.tile([128, NCH], F32)
            nc.gpsimd.dma_start(beta_sb[:], beta_dr[:])
            vb_sb = singles.tile([128, NCH], F32)
            nc.gpsimd.dma_start(vb_sb[:], vb_dr[:])
            srep_sb = singles.tile([128, 32 * 8], F32)
            nc.gpsimd.dma_start(srep_sb[:], srep_dr[:])
            if add_qbias:
                qb_sb = singles.tile([128, 256], F32)
                nc.gpsimd.dma_start(qb_sb[:], qb_dr[:])
            if add_pbias:
                pb_sb = singles.tile([128, 256], F32)
                nc.gpsimd.dma_start(pb_sb[:], pb_dr[:])

            NBLK = N // 128            # 32 token chunks of 128

            def make_img_state(img):
                st = {}
                st["img"] = img
                st["xt"] = xt_pool.tile([128, NCH, H + 2, W + 2], F32, tag="xt",
                                        name=f"xt_{img}")
                nc.vector.memset(st["xt"][:, :, 0, :], 0.0)
                nc.vector.memset(st["xt"][:, :, H + 1, :], 0.0)
                nc.vector.memset(st["xt"][:, :, :, 0], 0.0)
                nc.vector.memset(st["xt"][:, :, :, W + 1], 0.0)
                st["yt"] = img_pool.tile([128, NCH, N], F32, tag="yt",
                                         name=f"yt{img}")
                st["vt"] = img_pool.tile([128, NCH, N], BF16, tag="vt",
                                         name=f"vt{img}")
                st["qk"] = img_pool.tile([128, NBLK, 512], F32, tag="qk",
                                         name=f"qk{img}")
                st["sq"] = img_pool.tile([128, NBLK, 16], F32, tag="sqall",
                                         name=f"sq{img}")
                st["w"] = img_pool.tile([128, NBLK, 8], F32, tag="wall",
                                        name=f"w{img}")
                st["att"] = [ps_attn.tile([128, 256], F32, tag="att",
                                          name=f"att{g}_{img}")
                             for g in range(2)]
                return st

            def load_transpose(st, tb):
                img = st["img"]
                stg = xstage.tile([128, 2, C], F32, name="stg")
                nc.sync.dma_start(
                    stg[:],
                    x_dr[img, tb * 256:(tb + 1) * 256, :].rearrange(
                        "(g p) c -> p g c", p=128),
                )
                tp = ps_mm.tile([128, 512], F32, tag="mm", name="tp")
                for g in range(2):
                    for cch in range(NCH):
                        nc.tensor.transpose(
                            tp[:, cch * 256 + g * 128:cch * 256 + g * 128 + 128],
                            stg[:, g, cch * 128:cch * 128 + 128],
                            ident_sb[:],
                        )
                r0 = tb * 4   # 256 tokens = 4 h-rows
                nc.scalar.copy(
                    _r(st["xt"][:, :, 1 + r0:1 + r0 + 4, 1:1 + W]),
                    tp[:],
                )

            def dwconv_block(st, hb):
                h0 = hb * HBLK
                for cch in range(NCH):
                    yp = ps_mm.tile([128, HBLK * W], F32, tag="mm", name="yp")
                    for ti, (dh, dw) in enumerate(TAPS):
                        nc.tensor.matmul(
                            yp[:],
                            _r(diag_sb[:, cch, ti, :]),
                            _r(st["xt"][:, cch, 1 + h0 + dh:1 + h0 + HBLK + dh,
                                         1 + dw:1 + W + dw]),
                            start=(ti == 0),
                            stop=(ti == len(TAPS) - 1),
                            skip_group_check=True,
                        )
                    nc.scalar.activation(
                        out=_r(st["yt"][:, cch, h0 * W:(h0 + HBLK) * W]),
                        in_=yp[:],
                        func=AF.Identity,
                        bias=beta_sb[:, cch:cch + 1],
                    )

            def v_block(st, nb):
                for vc in range(NCH):
                    vp = ps_mm.tile([128, 512], F32, tag="mm", name="vp")
                    for kc in range(NCH):
                        nc.tensor.matmul(
                            vp[:],
                            _r(pwv_sb[:, kc, vc, :]),
                            _r(st["yt"][:, kc, nb * 512:(nb + 1) * 512]),
                            start=(kc == 0),
                            stop=(kc == NCH - 1),
                        )
                    nc.scalar.activation(
                        out=st["vt"][:, vc, nb * 512:(nb + 1) * 512],
                        in_=vp[:],
                        func=AF.Identity,
                        bias=vb_sb[:, vc:vc + 1],
                    )

            def qk_block(st, t):
                qk = st["qk"]
                qp = ps_mm.tile([128, 512], F32, tag="mm", name="qp")
                for kc in range(NCH):
                    nc.tensor.matmul(
                        qp[:],
                        _r(st["yt"][:, kc, t * 128:(t + 1) * 128]),
                        _r(pwqk_sb[:, kc, :]),
                        start=(kc == 0),
                        stop=(kc == NCH - 1),
                    )
                nc.vector.tensor_copy(_r(qk[:, t, :]), qp[:])
                if add_qbias:
                    nc.vector.tensor_tensor(
                        out=_r(qk[:, t, 0:256]), in0=qk[:, t, 0:256],
                        in1=qb_sb[:], op=ALU.add,
                    )
                sq = sq_pool.tile([128, 512], F32, name="sq")
                if add_qbias:
                    nc.scalar.square(sq[:], qk[:, t, :])
                else:
                    nc.scalar.square(sq[:], qp[:])
                nc.vector.tensor_reduce(
                    out=st["sq"][:, t, :],
                    in_=sq.rearrange("p (g d) -> p g d", d=HD),
                    axis=mybir.AxisListType.X,
                    op=ALU.add,
                )

            def w_batch(st, b):
                # normalization weight for chunks [4b, 4b+4)
                sl = slice(4 * b, 4 * b + 4)
                w_all, sq_all = st["w"], st["sq"]
                nc.vector.tensor_tensor(
                    out=w_all[:, sl, :],
                    in0=sq_all[:, sl, 0:8],
                    in1=sq_all[:, sl, 8:16],
                    op=ALU.mult,
                )
                wf = w_all[:, sl, :].rearrange("p t h -> p (t h)")
                nc.vector.tensor_scalar(
                    out=wf, in0=wf, scalar1=float(L2_EPS * L2_EPS),
                    scalar2=None, op0=ALU.max,
                )
                nc.scalar.activation(wf, wf, AF.Sqrt)
                nc.vector.reciprocal(wf, wf)
                nc.vector.tensor_tensor(
                    out=wf, in0=wf, in1=srep_sb[:, 8 * 4 * b:8 * 4 * (b + 1)],
                    op=ALU.mult)

            def qs_mm2(st, t):
                qk = st["qk"]
                qs = qs_pool.tile([128, 256], F32, name="qs")
                nc.vector.tensor_tensor(
                    out=_r(qs[:].rearrange("p (h d) -> p h d", d=HD)),
                    in0=qk[:, t, 0:256].rearrange("p (h d) -> p h d", d=HD),
                    in1=st["w"][:, t, :].unsqueeze(2).broadcast_to([128, 8, HD]),
                    op=ALU.mult,
                )
                for g in range(2):
                    nc.tensor.matmul(
                        st["att"][g][:],
                        _r(qs[:, g * 128:(g + 1) * 128]),
                        _r(qk[:, t, 256:512]),
                        start=(t == 0),
                        stop=(t == NBLK - 1),
                    )

            def softmax_at(st):
                at_bd = small.tile([128, 2, 128], BF16, tag="atbd", name="at_bd")
                st["at_bd"] = at_bd
                for g in range(2):
                    asm = small.tile([128, 32], F32, tag="asm", name="asm")
                    for j in range(4):
                        h = 4 * g + j
                        nc.vector.tensor_copy(
                            asm[32 * j:32 * j + 32, :],
                            st["att"][g][32 * j:32 * j + 32, 32 * h:32 * h + 32],
                        )
                    mx = small.tile([128, 1], F32, tag="mx", name="mx")
                    nc.vector.tensor_reduce(
                        out=mx[:], in_=asm[:], axis=mybir.AxisListType.X,
                        op=ALU.max, negate=True)
                    nc.scalar.activation(asm[:], asm[:], AF.Exp, bias=mx[:])
                    sm = small.tile([128, 1], F32, tag="sm", name="sm")
                    nc.vector.tensor_reduce(
                        out=sm[:], in_=asm[:], axis=mybir.AxisListType.X,
                        op=ALU.add)
                    nc.vector.reciprocal(sm[:], sm[:])
                    nc.vector.tensor_scalar(
                        out=asm[:], in0=asm[:], scalar1=sm[:], scalar2=None,
                        op0=ALU.mult)
                    atf = small.tile([128, 128], F32, tag="atf", name="atf")
                    nc.vector.memset(atf[:], 0.0)
                    for j in range(4):
                        nc.vector.transpose(
                            atf[32 * j:32 * j + 32, 32 * j:32 * j + 32],
                            asm[32 * j:32 * j + 32, :],
                        )
                    nc.vector.tensor_copy(at_bd[:, g, :], atf[:])

            def c_block(st, nb):
                # one 512-token slab: attn^T @ v then proj + store
                img = st["img"]
                if nb == 0:
                    st["ocm"] = img_pool.tile([128, NCH, N], BF16, tag="ocm",
                                              name=f"ocm{img}")
                ocm = st["ocm"]
                for g in range(NCH):
                    op_ = ps_c.tile([128, 512], F32, tag="cmm", name="op_")
                    nc.tensor.matmul(
                        op_[:],
                        st["at_bd"][:, g, :],
                        st["vt"][:, g, nb * 512:(nb + 1) * 512],
                    )
                    if g == 0:
                        nc.vector.tensor_copy(
                            ocm[:, g, nb * 512:(nb + 1) * 512], op_[:])
                    else:
                        nc.scalar.copy(
                            ocm[:, g, nb * 512:(nb + 1) * 512], op_[:])
                for t in range(4 * nb, 4 * nb + 4):
                    pp = ps_c.tile([128, 256], F32, tag="cmm", name="pp")
                    for kc in range(NCH):
                        nc.tensor.matmul(
                            pp[:],
                            ocm[:, kc, t * 128:(t + 1) * 128],
                            projw_sb[:, kc, :],
                            start=(kc == 0),
                            stop=(kc == NCH - 1),
                        )
                    if t % 2 == 0:
                        ot = ostage.tile([128, 2, 256], F32, name="ot")
                        st["ot"] = ot
                    ot = st["ot"]
                    if add_pbias:
                        nc.vector.tensor_tensor(
                            out=ot[:, t % 2, :], in0=pp[:], in1=pb_sb[:],
                            op=ALU.add)
                    elif t % 2 == 0:
                        nc.scalar.copy(ot[:, t % 2, :], pp[:])
                    else:
                        nc.vector.tensor_copy(ot[:, t % 2, :], pp[:])
                    if t % 2 == 1:
                        nc.sync.dma_start(
                            out_dr[img, (t - 1) * 128:(t + 1) * 128,
                                   :].rearrange("(g p) c -> p g c", p=128),
                            ot[:],
                        )

            def phase_A(st, interleave=None):
                # interleave: optional callable(tb) emitting prev-img C blocks
                for tb in range(NBLK // 2):
                    load_transpose(st, tb)
                    if interleave is not None:
                        interleave(tb)
                    if tb >= 4 and tb % 2 == 0:
                        hb = (tb - 4) // 2
                        dwconv_block(st, hb)
                        if hb >= 2:
                            v_block(st, hb - 2)
                        for t in range(4 * hb, 4 * hb + 4):
                            qk_block(st, t)
                        w_batch(st, hb)
                        if hb >= 1:
                            for t in range(4 * (hb - 1), 4 * (hb - 1) + 4):
                                qs_mm2(st, t)
                for hb in (6, 7):
                    dwconv_block(st, hb)
                    v_block(st, hb - 2)
                    for t in range(4 * hb, 4 * hb + 4):
                        qk_block(st, t)
                    w_batch(st, hb)
                    for t in range(4 * (hb - 1), 4 * (hb - 1) + 4):
                        qs_mm2(st, t)
                v_block(st, 6)
                for t in range(28, 32):
                    qs_mm2(st, t)
                v_block(st, 7)
                softmax_at(st)

            import contextlib
            rep_engines = (mybir.EngineType.PE, mybir.EngineType.DVE,
                           mybir.EngineType.Activation, mybir.EngineType.SP,
                           mybir.EngineType.Pool)
            rep_ctx = (tc.For_i(0, reps, 1, hint_engines=rep_engines)
                       if reps > 1 else contextlib.nullcontext())
            with rep_ctx:
                prev = None
                for img in range(IMGS):
                    st = make_img_state(img)
                    if prev is None:
                        phase_A(st)
                    else:
                        pv = prev

                        def emit_c(tb, pv=pv):
                            if tb < 8:
                                c_block(pv, tb)
                        phase_A(st, interleave=emit_c)
                    prev = st
                for nb in range(8):
                    c_block(prev, nb)

    nc.finalize()
    return nc


def _prep_consts(dw_kernel, bn_gamma, bn_beta, pw_kernel, q_bias, v_bias,
                 scale, proj_w, proj_b):
    taps_w = np.empty((9, C), np.float32)
    for ti, (dh, dw) in enumerate(TAPS):
        taps_w[ti] = dw_kernel[dh + 1, dw + 1, 0, :] * bn_gamma

    diag = np.zeros((128, NCH, 9, 128), np.float32)
    idx = np.arange(128)
    for cch in range(NCH):
        for ti in range(9):
            diag[idx, cch, ti, idx] = taps_w[ti, cch * 128 + idx]

    pwqk = np.empty((128, NCH, 512), np.float32)
    pwv = np.empty((128, NCH, NCH, 128), np.float32)
    for kc in range(NCH):
        pwqk[:, kc, :] = pw_kernel[kc * 128:(kc + 1) * 128, 0:512]
        for vc in range(NCH):
            pwv[:, kc, vc, :] = pw_kernel[kc * 128:(kc + 1) * 128,
                                          512 + vc * 128:512 + (vc + 1) * 128]

    projw = np.empty((128, NCH, 256), ml_dtypes.bfloat16)
    for kc in range(NCH):
        projw[:, kc, :] = proj_w[kc * 128:(kc + 1) * 128, :].astype(ml_dtypes.bfloat16)

    s = np.exp(np.minimum(scale.reshape(HEADS), LOG_MAX_SCALE)).astype(np.float32)
    srep = np.tile(np.tile(s, 32)[None, :], (128, 1)).astype(np.float32)

    consts = {
        "diag": diag,
        "pwqk": pwqk,
        "pwv": pwv,
        "projw": projw,
        "ident": np.eye(128, dtype=np.float32),
        "beta": np.tile(bn_beta.reshape(NCH, 128).T[:, :], (1, 1)).astype(np.float32),
        "vb": v_bias.reshape(NCH, 128).T.astype(np.float32).copy(),
        "srep": srep,
        "qb": np.tile(q_bias[None, :], (128, 1)).astype(np.float32),
        "pb": np.tile(proj_b[None, :], (128, 1)).astype(np.float32),
    }
    # beta layout [128, NCH]
    consts["beta"] = bn_beta.reshape(NCH, 128).T.astype(np.float32).copy()
    return consts


def kernel(x, dw_kernel, bn_gamma, bn_beta, pw_kernel, q_bias, v_bias, scale,
           proj_w, proj_b):
    x = np.ascontiguousarray(np.asarray(x, np.float32))
    consts = _prep_consts(
        np.asarray(dw_kernel, np.float32), np.asarray(bn_gamma, np.float32),
        np.asarray(bn_beta, np.float32), np.asarray(pw_kernel, np.float32),
        np.asarray(q_bias, np.float32), np.asarray(v_bias, np.float32),
        np.asarray(scale, np.float32), np.asarray(proj_w, np.float32),
        np.asarray(proj_b, np.float32))

    add_qbias = bool(np.any(q_bias))
    add_pbias = bool(np.any(proj_b))
    nc = _build_program(consts, add_qbias, add_pbias)

    xs = x.reshape(NCORES, IMGS, N, C)
    in_maps = [{"x": np.ascontiguousarray(xs[i])} for i in range(NCORES)]
    res = run_bass_kernel_spmd(nc, in_maps, core_ids=list(range(NCORES)))
    out = np.stack([res.results[i]["out"] for i in range(NCORES)])
    return out.reshape(B, H, W, C)


if __name__ == "__main__":
    pass
